# revision 18
# baseline (speedup 1.0000x reference)
"""Trainium2 Bass kernel for CohereAttention (GQA + interleaved RoPE + causal).

Sharding: TP-8 over heads (4 q heads + 1 kv head per core), both batches on
every core. All weights persist in SBUF (loaded once, outside the rep loop);
per-iteration DMA is only hidden_states in and partial outputs out. The host
sums the 8 partial output projections (TP all-reduce on host).

Pipeline: per block i (8 blocks = 2 batches x 4 token-blocks of 512), the
attention of block i-1 (ACT/DVE-paced exp->mask->racc chains) is interleaved
at kt granularity with the GEMM1 matmuls of block i and the GEMM2 matmuls of
block i-2, so the TensorEngine never idles waiting on the softmax chain.
Softmax row-normalization uses gpsimd partition_all_reduce (rowsum+broadcast
in one Pool op) off the PE critical path.

Layouts (feature-major): qkv^T [768, T] = wq-tiles.T @ hid^T; RoPE via a
128x128 permutation matmul + cos/sin tables; S^T [k,q] = KT-tile.T @ Q^T;
exp on ACT; PV and out^T = wo-tiles.T @ attn^T on PE.
"""

import numpy as np
import ml_dtypes

import concourse.bass as bass
import concourse.tile as tile
from concourse import bacc, mybir
from concourse.bass_utils import run_bass_kernel_spmd
from concourse.masks import make_identity
from concourse import bass_isa

BF16 = ml_dtypes.bfloat16

B, S, H = 2, 2048, 4096
NH, NKV, D = 32, 8, 128
G = NH // NKV
THETA = 10000.0

# per-core shard (TP8): 4 q heads + 1 kv head
QH = 4
KH = 1
N1 = (QH + 2 * KH) * D   # 768 qkv columns per core
AO = QH * D              # 512 attn-out dims per core
TB = 512                 # token block
NTB = S // TB            # 4 blocks per batch
S2 = B * S               # tokens across both batches (col dim of hidT/outT)
NBLK = B * NTB           # 8 blocks total
NM1 = N1 // 128          # 6 GEMM1 m-tiles
NM2 = H // 128           # 32 GEMM2 m-tiles
NKC = H // 128           # 32 contraction tiles
NCH = 4                  # hid DMA chunks per block
KPC = NKC // NCH         # k-slices per chunk
SCALE = float(D) ** -0.5


def _pin_activation_tables(arch):
    """Narrow bacc's view of the ACT function-table sets so Exp and Ln can
    only be satisfied by the combined `natural_log_exp_and_others` set.
    Without this the compiler alternates exp_and_others <-> natural_log per
    attention head (~65 table loads/rep at ~1.3us each, serializing ACT).
    The set actually loaded really contains Exp+Ln+Copy, so numerics are
    unchanged -- this only steers the set chooser."""
    from concourse.hw_specs import get_activation_tables
    try:
        tabs = get_activation_tables(arch)
    except Exception:
        return
    combined = tabs.get("natural_log_exp_and_others")
    if not combined:
        return
    for name, fns in tabs.items():
        if name == "natural_log_exp_and_others":
            continue
        fns.discard(mybir.ActivationFunctionType.Exp)
        fns.discard(mybir.ActivationFunctionType.Ln)


def build_nc(reps=1):
    nc = bacc.Bacc("TRN2", target_bir_lowering=False, debug=False,
                   enable_asserts=False)
    _pin_activation_tables(nc.m.arch)
    dt = mybir.dt

    hidT = nc.dram_tensor("hidT", [128, H // 128, S2], dt.bfloat16,
                          kind="ExternalInput").ap()
    wq_d = nc.dram_tensor("wq_d", [NM1, 128, H], dt.bfloat16,
                          kind="ExternalInput").ap()
    wo_d = nc.dram_tensor("wo_d", [NM2, 128, AO], dt.bfloat16,
                          kind="ExternalInput").ap()
    cosE = nc.dram_tensor("cosE", [128, S2], dt.bfloat16, kind="ExternalInput").ap()
    sinE = nc.dram_tensor("sinE", [128, S2], dt.bfloat16, kind="ExternalInput").ap()
    maskd = nc.dram_tensor("maskd", [128, TB], dt.bfloat16,
                           kind="ExternalInput").ap()
    rotmd = nc.dram_tensor("rotmd", [128, 128], dt.bfloat16,
                           kind="ExternalInput").ap()
    outT = nc.dram_tensor("outT", [H, S2], dt.bfloat16, kind="ExternalOutput").ap()

    with tile.TileContext(nc) as tc:
        with (
            tc.tile_pool(name="const", bufs=1) as const,
            tc.tile_pool(name="wq", bufs=1) as wq_pool,
            tc.tile_pool(name="wo", bufs=1) as wo_pool,
            tc.tile_pool(name="persist", bufs=1) as persist,
            tc.tile_pool(name="hid", bufs=2) as hid_pool,
            tc.tile_pool(name="cs", bufs=2) as cs_pool,
            tc.tile_pool(name="pre", bufs=1) as pre_pool,
            tc.tile_pool(name="qrope", bufs=2) as qrope_pool,
            tc.tile_pool(name="probs", bufs=4) as probs_pool,
            tc.tile_pool(name="attnT", bufs=2) as attnT_pool,
            tc.tile_pool(name="tmp", bufs=1) as tmp_pool,
            tc.tile_pool(name="norm", bufs=2) as norm_pool,
            tc.tile_pool(name="ost", bufs=2) as ost_pool,
            tc.tile_pool(name="g1_ps", bufs=2, space="PSUM") as g1_ps,
            tc.tile_pool(name="g2_ps", bufs=2, space="PSUM") as g2_ps,
            tc.tile_pool(name="s_ps", bufs=2, space="PSUM") as s_ps_pool,
            tc.tile_pool(name="o_ps", bufs=2, space="PSUM") as o_ps_pool,
        ):
            # ---- constants & persistent weights (outside rep loop) ----
            ident = const.tile([128, 128], dt.bfloat16)
            make_identity(nc, ident)
            ones = const.tile([128, 128], dt.bfloat16)
            nc.vector.memset(ones[:], 1.0)
            rotm = const.tile([128, 128], dt.bfloat16)
            nc.sync.dma_start(rotm[:], rotmd)
            masks = const.tile([128, TB], dt.bfloat16)
            nc.sync.dma_start(masks[:], maskd)

            wq_sb = []
            for m in range(NM1):
                t = wq_pool.tile([128, H], dt.bfloat16, tag=f"wq{m}", name=f"wq{m}")
                nc.sync.dma_start(t[:], wq_d[m])
                wq_sb.append(t)
            wo_sb = []
            for m in range(NM2):
                t = wo_pool.tile([128, AO], dt.bfloat16, tag=f"wo{m}", name=f"wo{m}")
                nc.sync.dma_start(t[:], wo_d[m])
                wo_sb.append(t)

            # persistent K^T (feature-major) and V (token-major) caches, per batch
            KTp = [persist.tile([128, S], dt.bfloat16, tag=f"KT{b}", name=f"KT{b}")
                   for b in range(B)]
            Vp = [persist.tile([128, S], dt.bfloat16, tag=f"V{b}", name=f"V{b}")
                  for b in range(B)]

            blocks = [(b, tb) for b in range(B) for tb in range(NTB)]

            # per-block state handed from section i to section i+1
            state = {}

            def emit_hid_dma(i):
                # 4 chunked DMAs (1MB each) instead of one 4MB transfer: the
                # monolithic DMA blocked the SP queue ~12us and GEMM1 of the
                # block had to wait for the LAST byte; chunked, the first
                # m-chain starts after ~3us and overlaps the rest.
                b, tb = blocks[i]
                col0 = b * S + tb * TB
                chs = []
                for c in range(NCH):
                    t = hid_pool.tile([128, KPC, TB], dt.bfloat16,
                                      tag=f"hid{c}", name=f"hid{c}")
                    nc.sync.dma_start(
                        t[:], hidT[:, c * KPC:(c + 1) * KPC, col0:col0 + TB])
                    chs.append(t)
                state[("hid", i)] = chs
                ct = cs_pool.tile([128, TB], dt.bfloat16, tag="cos", name="cos")
                nc.sync.dma_start(ct[:], cosE[:, col0:col0 + TB])
                st = cs_pool.tile([128, TB], dt.bfloat16, tag="sin", name="sin")
                nc.sync.dma_start(st[:], sinE[:, col0:col0 + TB])
                state[("cs", i)] = (ct, st)

            def g1_fills(i):
                """Closures: GEMM1 matmuls (+ PSUM->SBUF copies) for block i."""
                hid = state[("hid", i)]
                pre = [None] * NM1
                state[("pre", i)] = pre
                ops = []
                for m in range(NM1):
                    ps_ref = []
                    for k in range(NKC):
                        def op(m=m, k=k, ps_ref=ps_ref):
                            if k == 0:
                                ps_ref.append(g1_ps.tile([128, TB], dt.float32,
                                                         tag="g1", name="g1ps"))
                            ps = ps_ref[0]
                            nc.tensor.matmul(ps[:],
                                             wq_sb[m][:, k * 128:(k + 1) * 128],
                                             hid[k // KPC][:, k % KPC, :],
                                             start=(k == 0),
                                             stop=(k == NKC - 1))
                            if k == NKC - 1:
                                pt = pre_pool.tile([128, TB], dt.bfloat16,
                                                   tag=f"pre{m}", name=f"pre{m}")
                                nc.vector.tensor_copy(pt[:], ps[:])
                                pre[m] = pt
                        ops.append(op)
                return ops

            def g2_fills(i):
                """Closures: GEMM2 matmuls + out copies/DMA for block i."""
                b, tb = blocks[i]
                col0 = b * S + tb * TB
                at = state.pop(("attnT", i))
                ops = []
                for m in range(NM2):
                    ps_ref = []
                    for a in range(QH):
                        def op(m=m, a=a, ps_ref=ps_ref, col0=col0, at=at):
                            if a == 0:
                                ps_ref.append(g2_ps.tile([128, TB], dt.float32,
                                                         tag="g2", name="g2ps"))
                            ps = ps_ref[0]
                            nc.tensor.matmul(ps[:],
                                             wo_sb[m][:, a * 128:(a + 1) * 128],
                                             at[a][:], start=(a == 0),
                                             stop=(a == QH - 1))
                            if a == QH - 1:
                                ot = ost_pool.tile([128, TB], dt.bfloat16, name="ost")
                                nc.scalar.copy(ot[:], ps[:])
                                nc.sync.dma_start(
                                    outT[m * 128:(m + 1) * 128, col0:col0 + TB],
                                    ot[:])
                        ops.append(op)
                return ops

            def attn_gen(i):
                """Generator emitting attention for block i as a 2-head
                wavefront: within each kt step the PE stream is
                [scores h0, scores h1, PV h0, PV h1], so one head's
                exp/mask chain latency is hidden behind the other head's
                matmuls and the PE pipeline never drains between dependent
                matmuls. Yields (3 per kt step) let the driver interleave
                fill matmuls."""
                b, tb = blocks[i]
                qT = state.pop(("qT", i))
                nkt = (tb + 1) * (TB // 128)
                atl = [None] * QH
                state[("attnT", i)] = atl
                for hp in range(QH // 2):
                    pair = (2 * hp, 2 * hp + 1)
                    ops = {}
                    racc = {}
                    for h in pair:
                        ops[h] = o_ps_pool.tile([128, TB], dt.float32,
                                                tag="ops", name="opsps")
                        racc[h] = norm_pool.tile([128, TB], dt.bfloat16,
                                                 tag="racc", name="racc")
                    for kt in range(nkt):
                        v = kt - (TB // 128) * tb
                        q0 = 128 * v if v > 0 else 0
                        N = TB - q0
                        pr = {}
                        for h in pair:
                            sps = s_ps_pool.tile([128, TB], dt.float32,
                                                 tag="sps", name="sps")
                            nc.tensor.matmul(sps[:, :N],
                                             KTp[b][:, kt * 128:(kt + 1) * 128],
                                             qT[h][:, q0:TB], start=True,
                                             stop=True)
                            p = probs_pool.tile([128, TB], dt.bfloat16,
                                                tag="probs", name="probs")
                            nc.scalar.activation(p[:, :N], sps[:, :N],
                                                 mybir.ActivationFunctionType.Exp,
                                                 scale=SCALE)
                            if v >= 0:
                                nc.vector.tensor_mul(p[:, :N], p[:, :N],
                                                     masks[:, :N])
                            pr[h] = p
                            yield
                        for h in pair:
                            nc.tensor.matmul(ops[h][:, q0:TB],
                                             Vp[b][:, kt * 128:(kt + 1) * 128],
                                             pr[h][:, :N], start=(kt == 0),
                                             stop=(kt == nkt - 1))
                            if kt == 0:
                                nc.vector.tensor_copy(racc[h][:], pr[h][:, :N])
                            else:
                                nc.vector.tensor_add(racc[h][:, q0:TB],
                                                     racc[h][:, q0:TB],
                                                     pr[h][:, :N])
                        yield
                    # normalization tail: broadcast rowsum via all-ones
                    # matmul (one 213ns PE op), then 1/r = exp(-ln r) on ACT
                    # (same activation-table set as the attention exp) —
                    # replaces the serial 7µs PartitionAllReduce+RECIPROCAL
                    # chain that stalled the PE.
                    for h in pair:
                        rs = s_ps_pool.tile([128, TB], dt.float32, tag="sps",
                                            name="rsps")
                        nc.tensor.matmul(rs[:], ones[:], racc[h][:],
                                         start=True, stop=True)
                        lr = norm_pool.tile([128, TB], dt.float32, tag="rb",
                                            name="lr")
                        nc.scalar.activation(lr[:], rs[:],
                                             mybir.ActivationFunctionType.Ln)
                        rc = norm_pool.tile([128, TB], dt.float32, tag="rc",
                                            name="rc")
                        nc.scalar.activation(rc[:], lr[:],
                                             mybir.ActivationFunctionType.Exp,
                                             scale=-1.0)
                        at = attnT_pool.tile([128, TB], dt.bfloat16,
                                             tag=f"at{h}", name=f"at{h}")
                        nc.vector.tensor_mul(at[:], ops[h][:], rc[:])
                        atl[h] = at

            def rope_and_v(i):
                b, tb = blocks[i]
                ktsl = slice(tb * TB, (tb + 1) * TB)
                pre = state.pop(("pre", i))
                cos_t, sin_t = state.pop(("cs", i))
                qT = [None] * QH
                state[("qT", i)] = qT
                for idx in range(QH + KH):
                    src = pre[idx]
                    rps = g1_ps.tile([128, TB], dt.float32, tag="g1", name="ropeps")
                    nc.tensor.matmul(rps[:], rotm[:], src[:], start=True,
                                     stop=True)
                    t1 = tmp_pool.tile([128, TB], dt.float32, tag="ropet1", name="ropet1")
                    nc.vector.tensor_mul(t1[:], src[:], cos_t[:])
                    t2 = tmp_pool.tile([128, TB], dt.float32, tag="ropet2", name="ropet2")
                    nc.vector.tensor_mul(t2[:], rps[:], sin_t[:])
                    if idx < QH:
                        dst = qrope_pool.tile([128, TB], dt.bfloat16,
                                              tag=f"q{idx}", name=f"q{idx}")
                        nc.vector.tensor_add(dst[:], t1[:], t2[:])
                        qT[idx] = dst
                    else:
                        nc.vector.tensor_add(KTp[b][:, ktsl], t1[:], t2[:])
                # V^T -> V (token-major) via PE transpose (PSUM slots shared
                # with the g2 tag so GEMM2 chains get double-buffering)
                vsrc = pre[QH + KH]
                for tt in range(TB // 128):
                    tp = g2_ps.tile([128, 128], dt.bfloat16, tag="g2", name="tpps")
                    nc.tensor.transpose(tp[:], vsrc[:, tt * 128:(tt + 1) * 128],
                                        ident[:])
                    kt_g = tb * (TB // 128) + tt
                    nc.scalar.copy(Vp[b][:, kt_g * 128:(kt_g + 1) * 128], tp[:])

            def drive(i_attn, fills):
                """Interleave attn(i_attn) kt groups with fill closures."""
                if i_attn is None:
                    for f in fills:
                        f()
                    return
                b, tb = blocks[i_attn]
                kts = (QH // 2) * (tb + 1) * (TB // 128) * 3
                gen = attn_gen(i_attn)
                emitted = 0
                # pre-emit a few fills so PE has work while the section
                # boundary's DVE backlog (rope of block i_attn) drains
                while emitted < min(24, len(fills)):
                    fills[emitted]()
                    emitted += 1
                step = 0
                while True:
                    try:
                        next(gen)
                    except StopIteration:
                        break
                    step += 1
                    want = (len(fills) * step) // kts
                    while emitted < want:
                        fills[emitted]()
                        emitted += 1
                while emitted < len(fills):
                    fills[emitted]()
                    emitted += 1

            # hint_engines prefetches the back-edge IRAM block (the body is
            # far larger than one 16KiB block per engine, so the branch
            # target I$-misses without it).
            rep_ctx = (tc.For_i(0, reps, 1,
                                hint_engines=tuple(mybir.ALL_ENGINES))
                       if reps > 1 else None)
            if rep_ctx is not None:
                rep_ctx.__enter__()

            emit_hid_dma(0)
            for i in range(NBLK):
                if i + 1 < NBLK:
                    emit_hid_dma(i + 1)
                fills = []
                if i >= 2:
                    fills += g2_fills(i - 2)
                fills += g1_fills(i)
                drive(i - 1 if i >= 1 else None, fills)
                rope_and_v(i)
            # drain: attention+GEMM2 of the last two blocks
            drive(NBLK - 1, g2_fills(NBLK - 2))
            drive(None, g2_fills(NBLK - 1))

            if rep_ctx is not None:
                rep_ctx.__exit__(None, None, None)

    nc.compile()
    return nc


build_nc_reps = True


_NC_CACHE = []


def _get_nc():
    if not _NC_CACHE:
        _NC_CACHE.append(build_nc())
    return _NC_CACHE[0]


def make_host_inputs(hidden_states, positions, w_qkv, w_o):
    """Build per-core input maps (8 cores: core t -> q heads 4t..4t+3,
    kv head t, both batches)."""
    inv_freq = 1.0 / (THETA ** (np.arange(0, D, 2, dtype=np.float64) / D))

    rotm = np.zeros((128, 128), np.float32)
    for i in range(64):
        rotm[2 * i, 2 * i + 1] = 1.0   # lhsT = R^T
        rotm[2 * i + 1, 2 * i] = -1.0
    rotm = rotm.astype(BF16)

    masks = np.zeros((128, TB), np.float32)
    j = np.arange(TB)
    ii = np.arange(128)
    masks[:, :] = (j[None, :] >= ii[:, None])
    masks = masks.astype(BF16)

    # hidden^T for both batches, tiled for one 3D-AP DMA per block:
    # hidT[p, k, c] = hidden^T[k*128 + p, c]
    hidT = np.concatenate([hidden_states[b].T for b in range(B)],
                          axis=1).astype(BF16)                  # [H, S2]
    hidT = np.ascontiguousarray(
        hidT.reshape(H // 128, 128, S2).transpose(1, 0, 2))     # [128, 32, S2]

    # cos/sin tables for both batches: [128, S2]
    pos = positions.astype(np.float64)                 # [B, S]
    freqs = pos[..., None] * inv_freq[None, None, :]   # [B, S, 64]
    cosE = np.concatenate(
        [np.repeat(np.cos(freqs[b]).T, 2, axis=0) for b in range(B)],
        axis=1).astype(BF16)                           # [128, S2]
    sinE = np.concatenate(
        [np.repeat(np.sin(freqs[b]).T, 2, axis=0) for b in range(B)],
        axis=1).astype(BF16)

    in_maps = []
    for t in range(8):
        qc = w_qkv[:, QH * t * D:(QH * t + QH) * D]
        kc = w_qkv[:, NH * D + t * D: NH * D + (t + 1) * D]
        vc = w_qkv[:, (NH + NKV) * D + t * D: (NH + NKV) * D + (t + 1) * D]
        wshard = np.concatenate([qc, kc, vc], axis=1).astype(BF16)  # [H, N1]
        wq_t = np.ascontiguousarray(
            wshard.reshape(H // 128, 128, N1 // 128, 128)
            .transpose(2, 1, 0, 3).reshape(N1 // 128, 128, H))

        wo_shard = w_o[AO * t:AO * (t + 1), :].astype(BF16)  # [AO, H]
        wo_t = np.ascontiguousarray(
            wo_shard.reshape(AO // 128, 128, H // 128, 128)
            .transpose(2, 1, 0, 3).reshape(H // 128, 128, AO))

        in_maps.append({
            "hidT": hidT, "wq_d": wq_t, "wo_d": wo_t,
            "cosE": cosE, "sinE": sinE, "maskd": masks, "rotmd": rotm,
        })
    return in_maps


def combine_outputs(results):
    out = np.zeros((B, S, H), np.float32)
    for core in range(8):
        o = results[core]["outT"].astype(np.float32)   # [H, S2]
        for b in range(B):
            out[b] += o[:, b * S:(b + 1) * S].T
    return out


def kernel(hidden_states, positions, w_qkv, w_o):
    hidden_states = np.asarray(hidden_states, dtype=np.float32)
    positions = np.asarray(positions)
    w_qkv = np.asarray(w_qkv, dtype=np.float32)
    w_o = np.asarray(w_o, dtype=np.float32)

    nc = _get_nc()
    in_maps = make_host_inputs(hidden_states, positions, w_qkv, w_o)
    res = run_bass_kernel_spmd(nc, in_maps, core_ids=list(range(8)))
    return combine_outputs(res.results)

